# revision 9
# baseline (speedup 1.0000x reference)
"""Qwen2.5-VL attention (mrope + GQA + causal mask + o_proj) on 8 Trainium2
NeuronCores.

Sharding: batch x query-chunk. Core c handles batch b = c//4 and query rows
[512*(c%4), 512*(c%4)+512). Each core computes K/V projections for all 2048
tokens of its batch, Q projection + full attention + o_proj for its 512 query
rows, and writes a [512, 2048] output slice. Host concatenates - no
cross-core reduction.

On-device layout: everything transposed so the PE contraction dim is always
on partitions.  Host pre-transposes hidden (xT), weights (wqT/wkT/wvT/woT),
merged-mrope cos/sin, and the mask slice.
  - QT/KT produced as [d, t]; scores computed transposed S^T[k, q]
  - exp on ScalarE with the 1/sqrt(D) scale folded in (mask pre-scaled by
    sqrt(D) on host); softmax denominators via ones-vector matmuls on PE
  - PV accumulates outT[d, q]; normalization via PE-broadcast reciprocal
  - o_proj consumes outT directly as lhsT

Matmuls run in fp32r (fp32 with 12-bit mantissa rounding, 4x faster than
plain fp32 on the PE).  Host pre-rounds all DMA-fed matmul operands; compute
ops that produce matmul operands write fp32r tiles (HW rounds on write).
"""

import sys

for _p in ("/opt/trn_rl_repo", "/root/.axon_site/_ro/trn_rl_repo"):
    if _p not in sys.path:
        sys.path.insert(0, _p)

import numpy as np

B = 2
S = 2048
HID = 2048
NH = 16
NKV = 2
D = 128
NQ = 512          # query rows per core
N_CORES = 8
SM_SCALE = 1.0 / np.sqrt(np.float32(D))
INV_SM_SCALE = float(np.sqrt(np.float32(D)))

_BUILD_CACHE = {}


def _round_fp32r(a):
    """Round-to-nearest-even to 12 explicit mantissa bits (fp32r)."""
    u = np.ascontiguousarray(a, np.float32).view(np.uint32)
    low = u & np.uint32(0xFFF)
    up = (u & np.uint32(0xFFFFF000)) + np.uint32(0x1000)
    half = low == np.uint32(0x800)
    rnd = np.where(low > 0x800, up,
                   np.where(half & ((u & np.uint32(0x1000)) != 0), up,
                            u & np.uint32(0xFFFFF000)))
    # keep NaN/Inf intact (exponent all ones)
    expmask = (u & np.uint32(0x7F800000)) == np.uint32(0x7F800000)
    rnd = np.where(expmask, u, rnd)
    return rnd.view(np.float32)


def _build_nc(mm="f32r"):
    import concourse.bass as bass
    import concourse.tile as tile
    from concourse import bacc, mybir

    F32 = mybir.dt.float32
    MMDT = mybir.dt.float32r if mm == "f32r" else F32

    nc = bacc.Bacc(target_bir_lowering=False, debug=False)

    def param(name, shape, dt=MMDT):
        return nc.declare_dram_parameter(name, list(shape), dt,
                                         isOutput=False)[:]

    xT = param("xT", [HID, S])
    xqT = param("xqT", [HID, NQ])
    wqT = param("wqT", [HID, HID])
    wkT = param("wkT", [HID, NKV * D])
    wvT = param("wvT", [HID, NKV * D])
    woT = param("woT", [HID, HID])
    bqT_d = param("bqT", [D, NH], F32)
    bkT_d = param("bkT", [D, NKV], F32)
    bv_d = param("bv", [1, NKV * D])
    cosT_d = param("cosT", [D, S])
    sinT_d = param("sinT", [D, S])
    cq_d = param("cosTq", [D, NQ])
    sq_d = param("sinTq", [D, NQ])
    # mask slice for this core's queries, transposed, pre-scaled by sqrt(D)
    maskT_d = param("maskT", [S, NQ], F32)
    out_d = nc.declare_dram_parameter("out", [NQ, HID], F32, isOutput=True)[:]

    HC = HID // 128   # 16 contraction chunks
    KT = S // 128     # 16 key tiles
    TC = S // NQ      # 4 token chunks (for K/V proj)
    QS = NQ // 128    # 4 query sub-tiles

    Exp = mybir.ActivationFunctionType.Exp
    Ident = mybir.ActivationFunctionType.Identity

    import contextlib
    lp = (nc.allow_low_precision(reason="fp32r matmul operands; psum stays f32")
          if mm == "f32r" else contextlib.nullcontext())
    with lp, tile.TileContext(nc) as tc:
        with tc.tile_pool(name="const", bufs=1) as cst, \
             tc.tile_pool(name="maskp", bufs=1) as maskp, \
             tc.tile_pool(name="kvp", bufs=1) as kvp, \
             tc.tile_pool(name="qtp", bufs=1) as qtp:

            ones_col = cst.tile([128, 1], MMDT, name="ones_col")
            ones_row = cst.tile([1, 128], MMDT, name="ones_row")
            ones_f32 = cst.tile([128, 1], F32, name="ones_f32")
            ones_f32r = cst.tile([1, 128], F32, name="ones_f32r")
            nc.vector.memset(ones_f32, 1.0)
            nc.vector.memset(ones_f32r, 1.0)
            nc.vector.tensor_copy(ones_col, ones_f32)
            nc.vector.tensor_copy(ones_row, ones_f32r)
            bqT = cst.tile([D, NH], F32, name="bqT")
            bkT = cst.tile([D, NKV], F32, name="bkT")
            bvr = cst.tile([1, NKV * D], MMDT, name="bvr")
            nc.sync.dma_start(bqT, bqT_d)
            nc.sync.dma_start(bkT, bkT_d)
            nc.sync.dma_start(bvr, bv_d)

            # mask tiles [128 k, 512 q], resident through P2
            mask_sb = []
            for kt in range(KT):
                mt = maskp.tile([128, NQ], F32, name=f"mask{kt}")
                nc.sync.dma_start(mt, maskT_d[kt * 128:(kt + 1) * 128, :])
                mask_sb.append(mt)

            # persistent K^T [d, t] per kv head; V [t, d] per token tile
            kT_sb = [kvp.tile([128, S], MMDT, name=f"kT{g}")
                     for g in range(NKV)]
            v_sb = [kvp.tile([128, NKV * D], MMDT, name=f"v{t}")
                    for t in range(KT)]
            # Q^T (roped) per head [d, q]
            qT_sb = [qtp.tile([128, NQ], MMDT, name=f"qT{h}")
                     for h in range(NH)]

            # ---------------- P1a: K/V projection over all tokens ----------
            with tc.tile_pool(name="p1", bufs=1) as p1, \
                 tc.tile_pool(name="p1s", bufs=3) as p1s, \
                 tc.tile_pool(name="p1ps", bufs=1, space="PSUM") as p1ps:
                wk_sb = [p1.tile([128, NKV * D], MMDT, name=f"wk{c}")
                         for c in range(HC)]
                wv_sb = [p1.tile([128, NKV * D], MMDT, name=f"wv{c}")
                         for c in range(HC)]
                for c in range(HC):
                    nc.sync.dma_start(wk_sb[c], wkT[c * 128:(c + 1) * 128, :])
                    nc.sync.dma_start(wv_sb[c], wvT[c * 128:(c + 1) * 128, :])

                for tch in range(TC):
                    tsl = slice(tch * NQ, (tch + 1) * NQ)
                    kps = [p1ps.tile([128, NQ], F32, name=f"kps{g}", bufs=2)
                           for g in range(NKV)]
                    vps = [p1ps.tile([128, NKV * D], F32, name=f"vps{s_}",
                                     bufs=1) for s_ in range(4)]
                    for c in range(HC):
                        xt = p1s.tile([128, NQ], MMDT, name="xt")
                        nc.sync.dma_start(xt, xT[c * 128:(c + 1) * 128, tsl])
                        for g in range(NKV):
                            nc.tensor.matmul(
                                kps[g], wk_sb[c][:, g * D:(g + 1) * D],
                                xt, start=(c == 0), stop=(c == HC - 1))
                        for s_ in range(4):
                            nc.tensor.matmul(
                                vps[s_], xt[:, s_ * 128:(s_ + 1) * 128],
                                wv_sb[c], start=(c == 0), stop=False)
                    # V bias via K=1 ones matmul, then evacuate
                    for s_ in range(4):
                        nc.tensor.matmul(vps[s_], ones_row, bvr,
                                         start=False, stop=True)
                        nc.vector.tensor_copy(v_sb[tch * 4 + s_], vps[s_])
                    # K bias + rope -> kT_sb
                    csb = p1s.tile([128, NQ], MMDT, name="csb")
                    ssb = p1s.tile([128, NQ], MMDT, name="ssb")
                    nc.sync.dma_start(csb, cosT_d[:, tsl])
                    nc.sync.dma_start(ssb, sinT_d[:, tsl])
                    for g in range(NKV):
                        kb = p1s.tile([128, NQ], MMDT, name="kb")
                        nc.scalar.activation(kb, kps[g], Ident,
                                             bias=bkT[:, g:g + 1])
                        ke = kT_sb[g][:, tsl]
                        shuf = p1s.tile([128, NQ], MMDT, name="shuf")
                        nc.sync.dma_start(shuf[0:64, :], kb[64:128, :])
                        nc.sync.dma_start(shuf[64:128, :], kb[0:64, :])
                        nc.vector.tensor_mul(ke, kb, csb)
                        nc.vector.tensor_mul(shuf, shuf, ssb)
                        nc.vector.tensor_add(ke, ke, shuf)

            # ---------------- P1b: Q projection for this core's rows -------
            with tc.tile_pool(name="p2", bufs=1) as p2, \
                 tc.tile_pool(name="p2s", bufs=3) as p2s, \
                 tc.tile_pool(name="p2w", bufs=3) as p2w, \
                 tc.tile_pool(name="p2ps", bufs=1, space="PSUM") as p2ps:
                xq_sb = [p2.tile([128, NQ], MMDT, name=f"xq{c}")
                         for c in range(HC)]
                for c in range(HC):
                    nc.sync.dma_start(xq_sb[c], xqT[c * 128:(c + 1) * 128, :])
                cq = p2.tile([D, NQ], MMDT, name="cq")
                sq = p2.tile([D, NQ], MMDT, name="sq")
                nc.sync.dma_start(cq, cq_d)
                nc.sync.dma_start(sq, sq_d)

                for hg in range(4):
                    qps = [p2ps.tile([128, NQ], F32, name=f"qps{j}", bufs=1)
                           for j in range(4)]
                    for c in range(HC):
                        wq = p2w.tile([128, NQ], MMDT, name="wq")
                        nc.sync.dma_start(
                            wq, wqT[c * 128:(c + 1) * 128,
                                    hg * NQ:(hg + 1) * NQ])
                        for j in range(4):
                            nc.tensor.matmul(
                                qps[j], wq[:, j * 128:(j + 1) * 128],
                                xq_sb[c], start=(c == 0),
                                stop=(c == HC - 1))
                    for j in range(4):
                        h = hg * 4 + j
                        qb = p2s.tile([128, NQ], MMDT, name="qb")
                        nc.scalar.activation(qb, qps[j], Ident,
                                             bias=bqT[:, h:h + 1])
                        qe = qT_sb[h]
                        shufq = p2s.tile([128, NQ], MMDT, name="shufq")
                        nc.sync.dma_start(shufq[0:64, :], qb[64:128, :])
                        nc.sync.dma_start(shufq[64:128, :], qb[0:64, :])
                        nc.vector.tensor_mul(qe, qb, cq)
                        nc.vector.tensor_mul(shufq, shufq, sq)
                        nc.vector.tensor_add(qe, qe, shufq)

            # ---------------- P2: attention ---------------------------------
            with tc.tile_pool(name="ap", bufs=1) as ap:
                a_sb = [ap.tile([128, NQ], MMDT, name=f"a{h}")
                        for h in range(NH)]
                with tc.tile_pool(name="att", bufs=20) as att, \
                     tc.tile_pool(name="atts", bufs=2) as atts, \
                     tc.tile_pool(name="attps", bufs=1, space="PSUM") as attps:
                    for h in range(NH):
                        g = h // (NH // NKV)
                        ops = attps.tile([128, NQ], F32, name="ops", bufs=2)
                        stats = attps.tile([1, NQ], F32, name="stats", bufs=2)
                        for kt in range(KT):
                            sps = attps.tile([128, NQ], F32, name="sps",
                                             bufs=2)
                            nc.tensor.matmul(
                                sps, kT_sb[g][:, kt * 128:(kt + 1) * 128],
                                qT_sb[h], start=True, stop=True)
                            nc.vector.tensor_add(sps, sps, mask_sb[kt])
                            ebuf = att.tile([128, NQ], MMDT, name="ebuf")
                            nc.scalar.activation(ebuf, sps, Exp,
                                                 scale=float(SM_SCALE))
                            nc.tensor.matmul(
                                stats, ones_col, ebuf,
                                start=(kt == 0), stop=(kt == KT - 1))
                            nc.tensor.matmul(
                                ops, v_sb[kt][:, g * D:(g + 1) * D], ebuf,
                                start=(kt == 0), stop=(kt == KT - 1))
                        recip = atts.tile([1, NQ], MMDT, name="recip")
                        nc.vector.reciprocal(recip, stats)
                        bca = attps.tile([128, NQ], F32, name="bca", bufs=2)
                        nc.tensor.matmul(bca, ones_row, recip,
                                         start=True, stop=True)
                        bcs = atts.tile([128, NQ], F32, name="bcs")
                        nc.vector.tensor_copy(bcs, bca)
                        nc.vector.tensor_mul(a_sb[h], ops, bcs)

                # ------------- P3: o_proj ----------------------------------
                with tc.tile_pool(name="wop", bufs=1) as wop, \
                     tc.tile_pool(name="wos", bufs=3) as wos, \
                     tc.tile_pool(name="wops", bufs=1, space="PSUM") as wops:
                    for ec in range(4):
                        wo_t = [wop.tile([128, NQ], MMDT, name=f"wo{h}",
                                         bufs=2) for h in range(NH)]
                        for h in range(NH):
                            nc.sync.dma_start(
                                wo_t[h], woT[h * 128:(h + 1) * 128,
                                             ec * NQ:(ec + 1) * NQ])
                        for qs_ in range(QS):
                            opo = wops.tile([128, NQ], F32, name="opo", bufs=3)
                            for h in range(NH):
                                nc.tensor.matmul(
                                    opo,
                                    a_sb[h][:, qs_ * 128:(qs_ + 1) * 128],
                                    wo_t[h], start=(h == 0),
                                    stop=(h == NH - 1))
                            osb = wos.tile([128, NQ], F32, name="osb")
                            nc.vector.tensor_copy(osb, opo)
                            nc.sync.dma_start(
                                out_d[qs_ * 128:(qs_ + 1) * 128,
                                      ec * NQ:(ec + 1) * NQ], osb)
    return nc


def get_nc(mm="f32r"):
    if mm not in _BUILD_CACHE:
        nc = _build_nc(mm)
        nc.finalize()
        _BUILD_CACHE[mm] = nc
    return _BUILD_CACHE[mm]


_MROPE_SECTION = [16, 24, 24]
_STREAM_IDX = np.concatenate(
    [np.full(n, i % 3, np.int64)
     for i, n in enumerate(_MROPE_SECTION * 2)])  # [128]


def _host_prep(hidden_states, cos, sin, attention_mask, Wq, bq, Wk, bk, Wv,
               bv, Wo, mm="f32r"):
    f = np.float32
    if mm == "f32r":
        rnd = _round_fp32r
    else:
        def rnd(a):
            return np.ascontiguousarray(a, f)
    hs = np.asarray(hidden_states, f)
    cos = np.asarray(cos, f)
    sin = np.asarray(sin, f)
    mask = np.asarray(attention_mask, f)
    ar = np.arange(D)

    shared = {
        "wqT": rnd(np.asarray(Wq, f).T),
        "wkT": rnd(np.asarray(Wk, f).T),
        "wvT": rnd(np.asarray(Wv, f).T),
        "woT": rnd(np.asarray(Wo, f).T),
        "bqT": np.ascontiguousarray(np.asarray(bq, f).reshape(NH, D).T),
        "bkT": np.ascontiguousarray(np.asarray(bk, f).reshape(NKV, D).T),
        "bv": rnd(np.asarray(bv, f).reshape(1, NKV * D)),
    }

    per_batch = []
    for b in range(B):
        xT = rnd(hs[b].T)
        cosT = rnd(cos[_STREAM_IDX, b, :, ar])  # [128, S]
        sinT = rnd(sin[_STREAM_IDX, b, :, ar])
        sinT[0:64, :] *= -1.0
        maskT = np.ascontiguousarray(
            np.clip(mask[b, 0].T.astype(np.float64) * INV_SM_SCALE,
                    -3.0e38, 3.0e38).astype(np.float32))
        per_batch.append((xT, cosT, sinT, maskT))

    in_maps = []
    for c in range(N_CORES):
        b, qc = divmod(c, N_CORES // B)
        xT, cosT, sinT, maskT = per_batch[b]
        qsl = slice(qc * NQ, (qc + 1) * NQ)
        m = dict(shared)
        m["xT"] = xT
        m["cosT"] = cosT
        m["sinT"] = sinT
        m["maskT"] = np.ascontiguousarray(maskT[:, qsl])
        m["xqT"] = np.ascontiguousarray(xT[:, qsl])
        m["cosTq"] = np.ascontiguousarray(cosT[:, qsl])
        m["sinTq"] = np.ascontiguousarray(sinT[:, qsl])
        in_maps.append(m)
    return in_maps


def kernel(hidden_states, cos, sin, attention_mask, Wq, bq, Wk, bk, Wv, bv,
           Wo, _trace=False, _mm="f32r"):
    from concourse.bass_utils import run_bass_kernel_spmd

    in_maps = _host_prep(hidden_states, cos, sin, attention_mask, Wq, bq, Wk,
                         bk, Wv, bv, Wo, mm=_mm)
    nc = get_nc(_mm)
    res = run_bass_kernel_spmd(nc, in_maps, list(range(N_CORES)),
                               trace=_trace)
    out = np.empty((B, S, HID), np.float32)
    for c in range(N_CORES):
        b, qc = divmod(c, N_CORES // B)
        out[b, qc * NQ:(qc + 1) * NQ, :] = res.results[c]["out"]
    kernel._last_results = res
    return out


# revision 11
# speedup vs baseline: 1.2377x; 1.2377x over previous
"""Qwen2.5-VL attention (mrope + GQA + causal mask + o_proj) on 8 Trainium2
NeuronCores.

Sharding: batch x query-chunk. Core c handles batch b = c//4 and query rows
[512*(c%4), 512*(c%4)+512). Each core computes K/V projections for all 2048
tokens of its batch, Q projection + full attention + o_proj for its 512 query
rows, and writes a [512, 2048] output slice. Host concatenates - no
cross-core reduction.

On-device layout: everything transposed so the PE contraction dim is always
on partitions.  Host pre-transposes hidden (xT), weights (wqT/wkT/wvT/woT),
merged-mrope cos/sin, and the mask slice.
  - QT/KT produced as [d, t]; scores computed transposed S^T[k, q]
  - exp on ScalarE with the 1/sqrt(D) scale folded in (mask pre-scaled by
    sqrt(D) on host); softmax denominators via ones-vector matmuls on PE
  - PV accumulates outT[d, q]; normalization via PE-broadcast reciprocal
  - o_proj consumes outT directly as lhsT

Matmuls run in fp32r (fp32 with 12-bit mantissa rounding, 4x faster than
plain fp32 on the PE).  Host pre-rounds all DMA-fed matmul operands; compute
ops that produce matmul operands write fp32r tiles (HW rounds on write).
"""

import sys

for _p in ("/opt/trn_rl_repo", "/root/.axon_site/_ro/trn_rl_repo"):
    if _p not in sys.path:
        sys.path.insert(0, _p)

import numpy as np

B = 2
S = 2048
HID = 2048
NH = 16
NKV = 2
D = 128
NQ = 512          # query rows per core
N_CORES = 8
SM_SCALE = 1.0 / np.sqrt(np.float32(D))
INV_SM_SCALE = float(np.sqrt(np.float32(D)))

_BUILD_CACHE = {}


def _round_fp32r(a):
    """Round-to-nearest-even to 12 explicit mantissa bits (fp32r)."""
    u = np.ascontiguousarray(a, np.float32).view(np.uint32)
    low = u & np.uint32(0xFFF)
    up = (u & np.uint32(0xFFFFF000)) + np.uint32(0x1000)
    half = low == np.uint32(0x800)
    rnd = np.where(low > 0x800, up,
                   np.where(half & ((u & np.uint32(0x1000)) != 0), up,
                            u & np.uint32(0xFFFFF000)))
    # keep NaN/Inf intact (exponent all ones)
    expmask = (u & np.uint32(0x7F800000)) == np.uint32(0x7F800000)
    rnd = np.where(expmask, u, rnd)
    return rnd.view(np.float32)


def _build_nc(mm="f32r"):
    import concourse.bass as bass
    import concourse.tile as tile
    from concourse import bacc, mybir

    F32 = mybir.dt.float32
    MMDT = mybir.dt.float32r if mm == "f32r" else F32

    nc = bacc.Bacc(target_bir_lowering=False, debug=False)

    def param(name, shape, dt=MMDT):
        return nc.declare_dram_parameter(name, list(shape), dt,
                                         isOutput=False)[:]

    xT = param("xT", [HID, S])
    xqT = param("xqT", [HID, NQ])
    wqT = param("wqT", [HID, HID])
    wkT = param("wkT", [HID, NKV * D])
    wvT = param("wvT", [HID, NKV * D])
    woT = param("woT", [HID, HID])
    bqT_d = param("bqT", [D, NH], F32)
    bkT_d = param("bkT", [D, NKV], F32)
    bv_d = param("bv", [1, NKV * D])
    cosT_d = param("cosT", [D, S])
    sinT_d = param("sinT", [D, S])
    cq_d = param("cosTq", [D, NQ])
    sq_d = param("sinTq", [D, NQ])
    # mask slice for this core's queries, transposed, pre-scaled by sqrt(D)
    maskT_d = param("maskT", [S, NQ])
    out_d = nc.declare_dram_parameter("out", [NQ, HID], F32, isOutput=True)[:]

    HC = HID // 128   # 16 contraction chunks
    KT = S // 128     # 16 key tiles
    TC = S // NQ      # 4 token chunks (for K/V proj)
    QS = NQ // 128    # 4 query sub-tiles

    Exp = mybir.ActivationFunctionType.Exp
    Ident = mybir.ActivationFunctionType.Identity

    import contextlib
    lp = (nc.allow_low_precision(reason="fp32r matmul operands; psum stays f32")
          if mm == "f32r" else contextlib.nullcontext())
    with lp, tile.TileContext(nc) as tc:
        with tc.tile_pool(name="const", bufs=1) as cst, \
             tc.tile_pool(name="maskp", bufs=1) as maskp, \
             tc.tile_pool(name="kvp", bufs=1) as kvp, \
             tc.tile_pool(name="qtp", bufs=1) as qtp:

            ones_col = cst.tile([128, 1], MMDT, name="ones_col")
            ones_row = cst.tile([1, 128], MMDT, name="ones_row")
            ones_sq = cst.tile([128, 128], MMDT, name="ones_sq")
            ones_f32 = cst.tile([128, 128], F32, name="ones_f32")
            nc.vector.memset(ones_f32, 1.0)
            nc.vector.tensor_copy(ones_col, ones_f32[:, 0:1])
            nc.vector.tensor_copy(ones_row, ones_f32[0:1, :])
            nc.vector.tensor_copy(ones_sq, ones_f32)
            bqT = cst.tile([D, NH], F32, name="bqT")
            bkT = cst.tile([D, NKV], F32, name="bkT")
            bvr = cst.tile([1, NKV * D], MMDT, name="bvr")
            nc.sync.dma_start(bqT, bqT_d)
            nc.sync.dma_start(bkT, bkT_d)
            nc.sync.dma_start(bvr, bv_d)

            # exp(mask) tiles [128 k, 512 q], resident through P2
            # (DMAs are issued just before P2 so startup DMA prioritizes P1)
            mask_sb = [maskp.tile([128, NQ], MMDT, name=f"mask{kt}")
                       for kt in range(KT)]

            # persistent K^T [d, t] per kv head; V [t, d] per token tile
            kT_sb = [kvp.tile([128, S], MMDT, name=f"kT{g}")
                     for g in range(NKV)]
            v_sb = [kvp.tile([128, NKV * D], MMDT, name=f"v{t}")
                    for t in range(KT)]
            # Q^T (roped) per head [d, q]
            qT_sb = [qtp.tile([128, NQ], MMDT, name=f"qT{h}")
                     for h in range(NH)]

            # ---------------- P1a: K/V projection over all tokens ----------
            with tc.tile_pool(name="p1", bufs=1) as p1, \
                 tc.tile_pool(name="p1s", bufs=3) as p1s, \
                 tc.tile_pool(name="p1ps", bufs=1, space="PSUM") as p1ps:
                wk_sb = [p1.tile([128, NKV * D], MMDT, name=f"wk{c}")
                         for c in range(HC)]
                wv_sb = [p1.tile([128, NKV * D], MMDT, name=f"wv{c}")
                         for c in range(HC)]
                for c in range(HC):
                    nc.sync.dma_start(wk_sb[c], wkT[c * 128:(c + 1) * 128, :])
                    nc.sync.dma_start(wv_sb[c], wvT[c * 128:(c + 1) * 128, :])

                for tch in range(TC):
                    tsl = slice(tch * NQ, (tch + 1) * NQ)
                    kps = [p1ps.tile([128, NQ], F32, name=f"kps{g}", bufs=2)
                           for g in range(NKV)]
                    vps = [p1ps.tile([128, NKV * D], F32, name=f"vps{s_}",
                                     bufs=1) for s_ in range(4)]
                    for c in range(HC):
                        xt = p1s.tile([128, NQ], MMDT, name="xt")
                        nc.sync.dma_start(xt, xT[c * 128:(c + 1) * 128, tsl])
                        for g in range(NKV):
                            nc.tensor.matmul(
                                kps[g], wk_sb[c][:, g * D:(g + 1) * D],
                                xt, start=(c == 0), stop=(c == HC - 1))
                        for s_ in range(4):
                            nc.tensor.matmul(
                                vps[s_], xt[:, s_ * 128:(s_ + 1) * 128],
                                wv_sb[c], start=(c == 0), stop=False)
                    # V bias via K=1 ones matmul, then evacuate
                    for s_ in range(4):
                        nc.tensor.matmul(vps[s_], ones_row, bvr,
                                         start=False, stop=True)
                        nc.vector.tensor_copy(v_sb[tch * 4 + s_], vps[s_])
                    # K bias + rope -> kT_sb
                    csb = p1s.tile([128, NQ], MMDT, name="csb")
                    ssb = p1s.tile([128, NQ], MMDT, name="ssb")
                    nc.sync.dma_start(csb, cosT_d[:, tsl])
                    nc.sync.dma_start(ssb, sinT_d[:, tsl])
                    for g in range(NKV):
                        kb = p1s.tile([128, NQ], MMDT, name="kb")
                        nc.scalar.activation(kb, kps[g], Ident,
                                             bias=bkT[:, g:g + 1])
                        ke = kT_sb[g][:, tsl]
                        shuf = p1s.tile([128, NQ], MMDT, name="shuf")
                        nc.sync.dma_start(shuf[0:64, :], kb[64:128, :])
                        nc.sync.dma_start(shuf[64:128, :], kb[0:64, :])
                        nc.vector.tensor_mul(ke, kb, csb)
                        nc.vector.tensor_mul(shuf, shuf, ssb)
                        nc.vector.tensor_add(ke, ke, shuf)

            # ---------------- P1b: Q projection for this core's rows -------
            with tc.tile_pool(name="p2", bufs=1) as p2, \
                 tc.tile_pool(name="p2s", bufs=3) as p2s, \
                 tc.tile_pool(name="p2w", bufs=3) as p2w, \
                 tc.tile_pool(name="p2ps", bufs=1, space="PSUM") as p2ps:
                xq_sb = [p2.tile([128, NQ], MMDT, name=f"xq{c}")
                         for c in range(HC)]
                for c in range(HC):
                    nc.sync.dma_start(xq_sb[c], xqT[c * 128:(c + 1) * 128, :])
                cq = p2.tile([D, NQ], MMDT, name="cq")
                sq = p2.tile([D, NQ], MMDT, name="sq")
                nc.sync.dma_start(cq, cq_d)
                nc.sync.dma_start(sq, sq_d)

                for hg in range(4):
                    qps = [p2ps.tile([128, NQ], F32, name=f"qps{j}", bufs=1)
                           for j in range(4)]
                    for c in range(HC):
                        wq = p2w.tile([128, NQ], MMDT, name="wq")
                        nc.sync.dma_start(
                            wq, wqT[c * 128:(c + 1) * 128,
                                    hg * NQ:(hg + 1) * NQ])
                        for j in range(4):
                            nc.tensor.matmul(
                                qps[j], wq[:, j * 128:(j + 1) * 128],
                                xq_sb[c], start=(c == 0),
                                stop=(c == HC - 1))
                    for j in range(4):
                        h = hg * 4 + j
                        qb = p2s.tile([128, NQ], MMDT, name="qb")
                        nc.scalar.activation(qb, qps[j], Ident,
                                             bias=bqT[:, h:h + 1])
                        qe = qT_sb[h]
                        shufq = p2s.tile([128, NQ], MMDT, name="shufq")
                        nc.sync.dma_start(shufq[0:64, :], qb[64:128, :])
                        nc.sync.dma_start(shufq[64:128, :], qb[0:64, :])
                        nc.vector.tensor_mul(qe, qb, cq)
                        nc.vector.tensor_mul(shufq, shufq, sq)
                        nc.vector.tensor_add(qe, qe, shufq)

            # ---------------- P2: attention ---------------------------------
            for kt in range(KT):
                nc.sync.dma_start(mask_sb[kt],
                                  maskT_d[kt * 128:(kt + 1) * 128, :])
            with tc.tile_pool(name="ap", bufs=1) as ap:
                a_sb = [ap.tile([128, NQ], MMDT, name=f"a{h}")
                        for h in range(NH)]
                with tc.tile_pool(name="att", bufs=20) as att, \
                     tc.tile_pool(name="atts", bufs=2) as atts, \
                     tc.tile_pool(name="attps", bufs=1, space="PSUM") as attps:
                    for h in range(NH):
                        g = h // (NH // NKV)
                        ops = attps.tile([128, NQ], F32, name="ops", bufs=2)
                        stats = attps.tile([128, NQ], F32, name="stats",
                                           bufs=2)
                        for kt in range(KT):
                            sps = attps.tile([128, NQ], F32, name="sps",
                                             bufs=4)
                            nc.tensor.matmul(
                                sps, kT_sb[g][:, kt * 128:(kt + 1) * 128],
                                qT_sb[h], start=True, stop=True)
                            ebuf = att.tile([128, NQ], MMDT, name="ebuf")
                            nc.scalar.activation(ebuf, sps, Exp,
                                                 scale=float(SM_SCALE))
                            nc.vector.tensor_mul(ebuf, ebuf, mask_sb[kt])
                            nc.tensor.matmul(
                                stats, ones_sq, ebuf,
                                start=(kt == 0), stop=(kt == KT - 1))
                            nc.tensor.matmul(
                                ops, v_sb[kt][:, g * D:(g + 1) * D], ebuf,
                                start=(kt == 0), stop=(kt == KT - 1))
                        recip = atts.tile([128, NQ], MMDT, name="recip")
                        nc.vector.reciprocal(recip, stats)
                        nc.vector.tensor_mul(a_sb[h], ops, recip)

                # ------------- P3: o_proj ----------------------------------
                with tc.tile_pool(name="wop", bufs=1) as wop, \
                     tc.tile_pool(name="wos", bufs=3) as wos, \
                     tc.tile_pool(name="wops", bufs=1, space="PSUM") as wops:
                    for ec in range(4):
                        wo_t = [wop.tile([128, NQ], MMDT, name=f"wo{h}",
                                         bufs=2) for h in range(NH)]
                        for h in range(NH):
                            nc.sync.dma_start(
                                wo_t[h], woT[h * 128:(h + 1) * 128,
                                             ec * NQ:(ec + 1) * NQ])
                        for qs_ in range(QS):
                            opo = wops.tile([128, NQ], F32, name="opo", bufs=3)
                            for h in range(NH):
                                nc.tensor.matmul(
                                    opo,
                                    a_sb[h][:, qs_ * 128:(qs_ + 1) * 128],
                                    wo_t[h], start=(h == 0),
                                    stop=(h == NH - 1))
                            osb = wos.tile([128, NQ], F32, name="osb")
                            nc.vector.tensor_copy(osb, opo)
                            nc.sync.dma_start(
                                out_d[qs_ * 128:(qs_ + 1) * 128,
                                      ec * NQ:(ec + 1) * NQ], osb)
    return nc


def get_nc(mm="f32r"):
    if mm not in _BUILD_CACHE:
        nc = _build_nc(mm)
        nc.finalize()
        _BUILD_CACHE[mm] = nc
    return _BUILD_CACHE[mm]


_MROPE_SECTION = [16, 24, 24]
_STREAM_IDX = np.concatenate(
    [np.full(n, i % 3, np.int64)
     for i, n in enumerate(_MROPE_SECTION * 2)])  # [128]


def _host_prep(hidden_states, cos, sin, attention_mask, Wq, bq, Wk, bk, Wv,
               bv, Wo, mm="f32r"):
    f = np.float32
    if mm == "f32r":
        rnd = _round_fp32r
    else:
        def rnd(a):
            return np.ascontiguousarray(a, f)
    hs = np.asarray(hidden_states, f)
    cos = np.asarray(cos, f)
    sin = np.asarray(sin, f)
    mask = np.asarray(attention_mask, f)
    ar = np.arange(D)

    shared = {
        "wqT": rnd(np.asarray(Wq, f).T),
        "wkT": rnd(np.asarray(Wk, f).T),
        "wvT": rnd(np.asarray(Wv, f).T),
        "woT": rnd(np.asarray(Wo, f).T),
        "bqT": np.ascontiguousarray(np.asarray(bq, f).reshape(NH, D).T),
        "bkT": np.ascontiguousarray(np.asarray(bk, f).reshape(NKV, D).T),
        "bv": rnd(np.asarray(bv, f).reshape(1, NKV * D)),
    }

    per_batch = []
    for b in range(B):
        xT = rnd(hs[b].T)
        cosT = rnd(cos[_STREAM_IDX, b, :, ar])  # [128, S]
        sinT = rnd(sin[_STREAM_IDX, b, :, ar])
        sinT[0:64, :] *= -1.0
        maskT = rnd(np.exp(mask[b, 0].T.astype(np.float64)
                           ).astype(np.float32))
        per_batch.append((xT, cosT, sinT, maskT))

    in_maps = []
    for c in range(N_CORES):
        b, qc = divmod(c, N_CORES // B)
        xT, cosT, sinT, maskT = per_batch[b]
        qsl = slice(qc * NQ, (qc + 1) * NQ)
        m = dict(shared)
        m["xT"] = xT
        m["cosT"] = cosT
        m["sinT"] = sinT
        m["maskT"] = np.ascontiguousarray(maskT[:, qsl])
        m["xqT"] = np.ascontiguousarray(xT[:, qsl])
        m["cosTq"] = np.ascontiguousarray(cosT[:, qsl])
        m["sinTq"] = np.ascontiguousarray(sinT[:, qsl])
        in_maps.append(m)
    return in_maps


def kernel(hidden_states, cos, sin, attention_mask, Wq, bq, Wk, bk, Wv, bv,
           Wo, _trace=False, _mm="f32r"):
    from concourse.bass_utils import run_bass_kernel_spmd

    in_maps = _host_prep(hidden_states, cos, sin, attention_mask, Wq, bq, Wk,
                         bk, Wv, bv, Wo, mm=_mm)
    nc = get_nc(_mm)
    res = run_bass_kernel_spmd(nc, in_maps, list(range(N_CORES)),
                               trace=_trace)
    out = np.empty((B, S, HID), np.float32)
    for c in range(N_CORES):
        b, qc = divmod(c, N_CORES // B)
        out[b, qc * NQ:(qc + 1) * NQ, :] = res.results[c]["out"]
    kernel._last_results = res
    return out


# revision 12
# speedup vs baseline: 1.3326x; 1.0767x over previous
"""Qwen2.5-VL attention (mrope + GQA + causal mask + o_proj) on 8 Trainium2
NeuronCores.

Sharding: batch x query-chunk. Core c handles batch b = c//4 and query rows
[512*(c%4), 512*(c%4)+512). Each core computes K/V projections for all 2048
tokens of its batch, Q projection + full attention + o_proj for its 512 query
rows, and writes a [512, 2048] output slice. Host concatenates - no
cross-core reduction.

On-device layout: everything transposed so the PE contraction dim is always
on partitions.  Host pre-transposes hidden (xT), weights (wqT/wkT/wvT/woT),
merged-mrope cos/sin, and the mask slice.
  - QT/KT produced as [d, t]; scores computed transposed S^T[k, q]
  - exp on ScalarE straight from PSUM with the 1/sqrt(D) scale folded in;
    additive mask applied as elementwise multiply by host-precomputed
    exp(mask) (exact 0/1 for a causal mask)
  - softmax denominators via ones[128,128] matmuls (sums arrive broadcast
    across partitions), normalization = reciprocal + multiply
  - PV accumulates outT[d, q]; o_proj consumes outT directly as lhsT
  - Q projection is interleaved with attention per head group so the wq
    weight stream hides behind attention compute

Matmuls run in fp32r (fp32 with 12-bit mantissa rounding, 4x faster than
plain fp32 on the PE).  Host pre-rounds all DMA-fed matmul operands; compute
ops that produce matmul operands write fp32r tiles (HW rounds on write).
"""

import sys

for _p in ("/opt/trn_rl_repo", "/root/.axon_site/_ro/trn_rl_repo"):
    if _p not in sys.path:
        sys.path.insert(0, _p)

import numpy as np

B = 2
S = 2048
HID = 2048
NH = 16
NKV = 2
D = 128
NQ = 512          # query rows per core
N_CORES = 8
SM_SCALE = 1.0 / np.sqrt(np.float32(D))

_BUILD_CACHE = {}


def _round_fp32r(a):
    """Round-to-nearest-even to 12 explicit mantissa bits (fp32r)."""
    u = np.ascontiguousarray(a, np.float32).view(np.uint32)
    low = u & np.uint32(0xFFF)
    up = (u & np.uint32(0xFFFFF000)) + np.uint32(0x1000)
    half = low == np.uint32(0x800)
    rnd = np.where(low > 0x800, up,
                   np.where(half & ((u & np.uint32(0x1000)) != 0), up,
                            u & np.uint32(0xFFFFF000)))
    expmask = (u & np.uint32(0x7F800000)) == np.uint32(0x7F800000)
    rnd = np.where(expmask, u, rnd)
    return rnd.view(np.float32)


def _build_nc(mm="f32r"):
    import contextlib
    import concourse.bass as bass
    import concourse.tile as tile
    from concourse import bacc, mybir

    F32 = mybir.dt.float32
    MMDT = mybir.dt.float32r if mm == "f32r" else F32

    nc = bacc.Bacc(target_bir_lowering=False, debug=False)

    def param(name, shape, dt=MMDT):
        return nc.declare_dram_parameter(name, list(shape), dt,
                                         isOutput=False)[:]

    xT = param("xT", [HID, S])
    xqT = param("xqT", [HID, NQ])
    wqT = param("wqT", [HID, HID])
    wkT = param("wkT", [HID, NKV * D])
    wvT = param("wvT", [HID, NKV * D])
    woT = param("woT", [HID, HID])
    bqT_d = param("bqT", [D, NH], F32)
    bkT_d = param("bkT", [D, NKV], F32)
    bv_d = param("bv", [1, NKV * D])
    cosT_d = param("cosT", [D, S])
    sinT_d = param("sinT", [D, S])
    cq_d = param("cosTq", [D, NQ])
    sq_d = param("sinTq", [D, NQ])
    maskT_d = param("maskT", [S, NQ])     # exp(mask).T, fp32r-rounded
    out_d = nc.declare_dram_parameter("out", [NQ, HID], F32, isOutput=True)[:]

    HC = HID // 128   # 16 contraction chunks
    KT = S // 128     # 16 key tiles
    KT2 = KT // 2     # 8 key tile-pairs
    TC = S // NQ      # 4 token chunks (for K/V proj)
    QS = NQ // 128    # 4 query sub-tiles

    Exp = mybir.ActivationFunctionType.Exp
    Ident = mybir.ActivationFunctionType.Identity

    lp = (nc.allow_low_precision(reason="fp32r matmul operands; psum stays f32")
          if mm == "f32r" else contextlib.nullcontext())
    with lp, tile.TileContext(nc) as tc:
        with tc.tile_pool(name="const", bufs=1) as cst, \
             tc.tile_pool(name="maskp", bufs=1) as maskp, \
             tc.tile_pool(name="kvp", bufs=1) as kvp:

            ones_row = cst.tile([1, 128], MMDT, name="ones_row")
            ones_sq = cst.tile([128, 128], MMDT, name="ones_sq")
            ones_f32 = cst.tile([128, 128], F32, name="ones_f32")
            nc.vector.memset(ones_f32, 1.0)
            nc.vector.tensor_copy(ones_row, ones_f32[0:1, :])
            nc.vector.tensor_copy(ones_sq, ones_f32)
            bqT = cst.tile([D, NH], F32, name="bqT")
            bkT = cst.tile([D, NKV], F32, name="bkT")
            bvr = cst.tile([1, NKV * D], MMDT, name="bvr")
            nc.sync.dma_start(bqT, bqT_d)
            nc.sync.dma_start(bkT, bkT_d)
            nc.sync.dma_start(bvr, bv_d)

            # exp(mask) tiles [128 k, 2 kt, 512 q], resident through attention
            mask_sb = [maskp.tile([128, 2, NQ], MMDT, name=f"mask{kt}")
                       for kt in range(KT2)]

            # persistent K^T [d, t] per kv head; V [t, d] per token tile
            kT_sb = [kvp.tile([128, S], MMDT, name=f"kT{g}")
                     for g in range(NKV)]
            v_sb = [kvp.tile([128, NKV * D], MMDT, name=f"v{t}")
                    for t in range(KT)]

            # ---------------- P1a: K/V projection over all tokens ----------
            with tc.tile_pool(name="p1", bufs=1) as p1, \
                 tc.tile_pool(name="p1s", bufs=3) as p1s, \
                 tc.tile_pool(name="p1ps", bufs=1, space="PSUM") as p1ps:
                wk_sb = [p1.tile([128, NKV * D], MMDT, name=f"wk{c}")
                         for c in range(HC)]
                wv_sb = [p1.tile([128, NKV * D], MMDT, name=f"wv{c}")
                         for c in range(HC)]
                for c in range(HC):
                    nc.sync.dma_start(wk_sb[c], wkT[c * 128:(c + 1) * 128, :])
                    nc.sync.dma_start(wv_sb[c], wvT[c * 128:(c + 1) * 128, :])

                for tch in range(TC):
                    tsl = slice(tch * NQ, (tch + 1) * NQ)
                    kps = [p1ps.tile([128, NQ], F32, name=f"kps{g}", bufs=2)
                           for g in range(NKV)]
                    vps = [p1ps.tile([128, NKV * D], F32, name=f"vps{s_}",
                                     bufs=1) for s_ in range(4)]
                    for c in range(HC):
                        xt = p1s.tile([128, NQ], MMDT, name="xt")
                        nc.sync.dma_start(xt, xT[c * 128:(c + 1) * 128, tsl])
                        for g in range(NKV):
                            nc.tensor.matmul(
                                kps[g], wk_sb[c][:, g * D:(g + 1) * D],
                                xt, start=(c == 0), stop=(c == HC - 1))
                        for s_ in range(4):
                            nc.tensor.matmul(
                                vps[s_], xt[:, s_ * 128:(s_ + 1) * 128],
                                wv_sb[c], start=(c == 0), stop=False)
                    # V bias via K=1 ones matmul, then evacuate
                    for s_ in range(4):
                        nc.tensor.matmul(vps[s_], ones_row, bvr,
                                         start=False, stop=True)
                        nc.vector.tensor_copy(v_sb[tch * 4 + s_], vps[s_])
                    # K bias + rope -> kT_sb
                    csb = p1s.tile([128, NQ], MMDT, name="csb")
                    ssb = p1s.tile([128, NQ], MMDT, name="ssb")
                    nc.sync.dma_start(csb, cosT_d[:, tsl])
                    nc.sync.dma_start(ssb, sinT_d[:, tsl])
                    for g in range(NKV):
                        kb = p1s.tile([128, NQ], MMDT, name="kb")
                        nc.scalar.activation(kb, kps[g], Ident,
                                             bias=bkT[:, g:g + 1])
                        ke = kT_sb[g][:, tsl]
                        shuf = p1s.tile([128, NQ], MMDT, name="shuf")
                        nc.sync.dma_start(shuf[0:64, :], kb[64:128, :])
                        nc.sync.dma_start(shuf[64:128, :], kb[0:64, :])
                        nc.vector.tensor_mul(ke, kb, csb)
                        nc.vector.tensor_mul(shuf, shuf, ssb)
                        nc.vector.tensor_add(ke, ke, shuf)

            # -------- P1b + P2: Q proj interleaved with attention ----------
            for kt2 in range(KT2):
                nc.sync.dma_start(
                    mask_sb[kt2],
                    maskT_d[256 * kt2:256 * (kt2 + 1), :].rearrange(
                        "(a p) q -> p a q", a=2))
            with tc.tile_pool(name="ap", bufs=1) as ap:
                a_sb = [ap.tile([128, NQ], MMDT, name=f"a{h}")
                        for h in range(NH)]
                with tc.tile_pool(name="p2", bufs=1) as p2, \
                     tc.tile_pool(name="p2s", bufs=2) as p2s, \
                     tc.tile_pool(name="p2w", bufs=5) as p2w, \
                     tc.tile_pool(name="qtp", bufs=2) as qtp, \
                     tc.tile_pool(name="att", bufs=5) as att, \
                     tc.tile_pool(name="atts", bufs=2) as atts:
                    xq_sb = [p2.tile([128, NQ], MMDT, name=f"xq{c}")
                             for c in range(HC)]
                    for c in range(HC):
                        nc.sync.dma_start(xq_sb[c],
                                          xqT[c * 128:(c + 1) * 128, :])
                    cq = p2.tile([D, NQ], MMDT, name="cq")
                    sq = p2.tile([D, NQ], MMDT, name="sq")
                    nc.sync.dma_start(cq, cq_d)
                    nc.sync.dma_start(sq, sq_d)

                    for hg in range(4):
                        qT_sb = {}
                        with tc.tile_pool(name=f"qps{hg}", bufs=1,
                                          space="PSUM") as p2ps:
                            qps = [p2ps.tile([128, NQ], F32, name=f"qps{j}",
                                             bufs=1) for j in range(4)]
                            for c in range(HC):
                                wq = p2w.tile([128, NQ], MMDT, name="wq")
                                nc.sync.dma_start(
                                    wq, wqT[c * 128:(c + 1) * 128,
                                            hg * NQ:(hg + 1) * NQ])
                                for j in range(4):
                                    nc.tensor.matmul(
                                        qps[j], wq[:, j * 128:(j + 1) * 128],
                                        xq_sb[c], start=(c == 0),
                                        stop=(c == HC - 1))
                            for j in range(4):
                                h = hg * 4 + j
                                qT_sb[h] = qtp.tile([128, NQ], MMDT,
                                                    name=f"qT{j}")
                                qb = p2s.tile([128, NQ], MMDT, name="qb")
                                nc.scalar.activation(qb, qps[j], Ident,
                                                     bias=bqT[:, h:h + 1])
                                qe = qT_sb[h]
                                shufq = p2s.tile([128, NQ], MMDT,
                                                 name="shufq")
                                nc.sync.dma_start(shufq[0:64, :],
                                                  qb[64:128, :])
                                nc.sync.dma_start(shufq[64:128, :],
                                                  qb[0:64, :])
                                nc.vector.tensor_mul(qe, qb, cq)
                                nc.vector.tensor_mul(shufq, shufq, sq)
                                nc.vector.tensor_add(qe, qe, shufq)

                        with tc.tile_pool(name=f"attps{hg}", bufs=1,
                                          space="PSUM") as attps:
                            for h in range(hg * 4, hg * 4 + 4):
                                g = h // (NH // NKV)
                                ops = attps.tile([128, NQ], F32, name="ops",
                                                 bufs=2)
                                stats = attps.tile([128, NQ], F32,
                                                   name="stats", bufs=2)
                                for kt2 in range(KT2):
                                    sps = attps.tile([128, 2, NQ], F32,
                                                     name="sps", bufs=2)
                                    ebuf = att.tile([128, 2, NQ], MMDT,
                                                    name="ebuf")
                                    for j2 in range(2):
                                        kt = 2 * kt2 + j2
                                        nc.tensor.matmul(
                                            sps[:, j2, :],
                                            kT_sb[g][:, kt * 128:
                                                     (kt + 1) * 128],
                                            qT_sb[h], start=True, stop=True)
                                    nc.scalar.activation(
                                        ebuf.rearrange("p a b -> p (a b)"),
                                        sps.rearrange("p a b -> p (a b)"),
                                        Exp, scale=float(SM_SCALE))
                                    nc.vector.tensor_mul(
                                        ebuf.rearrange("p a b -> p (a b)"),
                                        ebuf.rearrange("p a b -> p (a b)"),
                                        mask_sb[kt2].rearrange(
                                            "p a b -> p (a b)"))
                                    for j2 in range(2):
                                        kt = 2 * kt2 + j2
                                        nc.tensor.matmul(
                                            stats, ones_sq, ebuf[:, j2, :],
                                            start=(kt == 0),
                                            stop=(kt == KT - 1))
                                        nc.tensor.matmul(
                                            ops,
                                            v_sb[kt][:, g * D:(g + 1) * D],
                                            ebuf[:, j2, :],
                                            start=(kt == 0),
                                            stop=(kt == KT - 1))
                                recip = atts.tile([128, NQ], MMDT,
                                                  name="recip")
                                nc.vector.reciprocal(recip, stats)
                                nc.vector.tensor_mul(a_sb[h], ops, recip)

                # ------------- P3: o_proj ------------------------------
                with tc.tile_pool(name="wop", bufs=1) as wop, \
                     tc.tile_pool(name="wos", bufs=3) as wos, \
                     tc.tile_pool(name="wops", bufs=1, space="PSUM") as wops:
                    for ec in range(4):
                        wo_t = [wop.tile([128, NQ], MMDT, name=f"wo{h}",
                                         bufs=2) for h in range(NH)]
                        for h in range(NH):
                            nc.sync.dma_start(
                                wo_t[h], woT[h * 128:(h + 1) * 128,
                                             ec * NQ:(ec + 1) * NQ])
                        for qs_ in range(QS):
                            opo = wops.tile([128, NQ], F32, name="opo",
                                            bufs=3)
                            for h in range(NH):
                                nc.tensor.matmul(
                                    opo,
                                    a_sb[h][:, qs_ * 128:(qs_ + 1) * 128],
                                    wo_t[h], start=(h == 0),
                                    stop=(h == NH - 1))
                            osb = wos.tile([128, NQ], F32, name="osb")
                            nc.vector.tensor_copy(osb, opo)
                            nc.sync.dma_start(
                                out_d[qs_ * 128:(qs_ + 1) * 128,
                                      ec * NQ:(ec + 1) * NQ], osb)
    return nc


def get_nc(mm="f32r"):
    if mm not in _BUILD_CACHE:
        nc = _build_nc(mm)
        nc.finalize()
        _BUILD_CACHE[mm] = nc
    return _BUILD_CACHE[mm]


_MROPE_SECTION = [16, 24, 24]
_STREAM_IDX = np.concatenate(
    [np.full(n, i % 3, np.int64)
     for i, n in enumerate(_MROPE_SECTION * 2)])  # [128]


def _host_prep(hidden_states, cos, sin, attention_mask, Wq, bq, Wk, bk, Wv,
               bv, Wo, mm="f32r"):
    f = np.float32
    if mm == "f32r":
        rnd = _round_fp32r
    else:
        def rnd(a):
            return np.ascontiguousarray(a, f)
    hs = np.asarray(hidden_states, f)
    cos = np.asarray(cos, f)
    sin = np.asarray(sin, f)
    mask = np.asarray(attention_mask, f)
    ar = np.arange(D)

    shared = {
        "wqT": rnd(np.asarray(Wq, f).T),
        "wkT": rnd(np.asarray(Wk, f).T),
        "wvT": rnd(np.asarray(Wv, f).T),
        "woT": rnd(np.asarray(Wo, f).T),
        "bqT": np.ascontiguousarray(np.asarray(bq, f).reshape(NH, D).T),
        "bkT": np.ascontiguousarray(np.asarray(bk, f).reshape(NKV, D).T),
        "bv": rnd(np.asarray(bv, f).reshape(1, NKV * D)),
    }

    per_batch = []
    for b in range(B):
        xT = rnd(hs[b].T)
        cosT = rnd(cos[_STREAM_IDX, b, :, ar])  # [128, S]
        sinT = rnd(sin[_STREAM_IDX, b, :, ar])
        sinT[0:64, :] *= -1.0   # rotate_half sign folded into sin
        maskT = rnd(np.exp(mask[b, 0].T.astype(np.float64)
                           ).astype(np.float32))
        per_batch.append((xT, cosT, sinT, maskT))

    in_maps = []
    for c in range(N_CORES):
        b, qc = divmod(c, N_CORES // B)
        xT, cosT, sinT, maskT = per_batch[b]
        qsl = slice(qc * NQ, (qc + 1) * NQ)
        m = dict(shared)
        m["xT"] = xT
        m["cosT"] = cosT
        m["sinT"] = sinT
        m["maskT"] = np.ascontiguousarray(maskT[:, qsl])
        m["xqT"] = np.ascontiguousarray(xT[:, qsl])
        m["cosTq"] = np.ascontiguousarray(cosT[:, qsl])
        m["sinTq"] = np.ascontiguousarray(sinT[:, qsl])
        in_maps.append(m)
    return in_maps


def kernel(hidden_states, cos, sin, attention_mask, Wq, bq, Wk, bk, Wv, bv,
           Wo, _trace=False, _mm="f32r"):
    from concourse.bass_utils import run_bass_kernel_spmd

    in_maps = _host_prep(hidden_states, cos, sin, attention_mask, Wq, bq, Wk,
                         bk, Wv, bv, Wo, mm=_mm)
    nc = get_nc(_mm)
    res = run_bass_kernel_spmd(nc, in_maps, list(range(N_CORES)),
                               trace=_trace)
    out = np.empty((B, S, HID), np.float32)
    for c in range(N_CORES):
        b, qc = divmod(c, N_CORES // B)
        out[b, qc * NQ:(qc + 1) * NQ, :] = res.results[c]["out"]
    kernel._last_results = res
    return out


# revision 13
# speedup vs baseline: 1.4816x; 1.1118x over previous
"""Qwen2.5-VL attention (mrope + GQA + causal mask + o_proj) on 8 Trainium2
NeuronCores.

Sharding: batch x query-chunk. Core c handles batch b = c//4 and query rows
[512*(c%4), 512*(c%4)+512). Each core computes K/V projections for all 2048
tokens of its batch, Q projection + full attention + o_proj for its 512 query
rows, and writes a [512, 2048] output slice. Host concatenates - no
cross-core reduction.

On-device layout: everything transposed so the PE contraction dim is always
on partitions.  Host pre-transposes hidden (xT), weights (wqT/wkT/wvT/woT),
merged-mrope cos/sin, and the mask slice.
  - QT/KT produced as [d, t]; scores computed transposed S^T[k, q]
  - exp on ScalarE straight from PSUM with the 1/sqrt(D) scale folded in;
    additive mask applied as elementwise multiply by host-precomputed
    exp(mask) (exact 0/1 for a causal mask)
  - softmax denominators via ones[128,128] matmuls (sums arrive broadcast
    across partitions), normalization = reciprocal + multiply
  - PV accumulates outT[d, q]; o_proj consumes outT directly as lhsT
  - Q projection is interleaved with attention per head group so the wq
    weight stream hides behind attention compute

Matmuls run in fp32r (fp32 with 12-bit mantissa rounding, 4x faster than
plain fp32 on the PE).  Host pre-rounds all DMA-fed matmul operands; compute
ops that produce matmul operands write fp32r tiles (HW rounds on write).
"""

import sys

for _p in ("/opt/trn_rl_repo", "/root/.axon_site/_ro/trn_rl_repo"):
    if _p not in sys.path:
        sys.path.insert(0, _p)

import numpy as np

B = 2
S = 2048
HID = 2048
NH = 16
NKV = 2
D = 128
NQ = 512          # query rows per core
N_CORES = 8
SM_SCALE = 1.0 / np.sqrt(np.float32(D))

_BUILD_CACHE = {}


def _round_fp32r(a):
    """Round-to-nearest-even to 12 explicit mantissa bits (fp32r)."""
    u = np.ascontiguousarray(a, np.float32).view(np.uint32)
    low = u & np.uint32(0xFFF)
    up = (u & np.uint32(0xFFFFF000)) + np.uint32(0x1000)
    half = low == np.uint32(0x800)
    rnd = np.where(low > 0x800, up,
                   np.where(half & ((u & np.uint32(0x1000)) != 0), up,
                            u & np.uint32(0xFFFFF000)))
    expmask = (u & np.uint32(0x7F800000)) == np.uint32(0x7F800000)
    rnd = np.where(expmask, u, rnd)
    return rnd.view(np.float32)


def _build_nc(mm="f32r"):
    import contextlib
    import concourse.bass as bass
    import concourse.tile as tile
    from concourse import bacc, mybir

    F32 = mybir.dt.float32
    MMDT = mybir.dt.float32r if mm == "f32r" else F32

    nc = bacc.Bacc(target_bir_lowering=False, debug=False)

    def param(name, shape, dt=MMDT):
        return nc.declare_dram_parameter(name, list(shape), dt,
                                         isOutput=False)[:]

    xT = param("xT", [HID, S])
    wqT = param("wqT", [HID, HID])
    wkT = param("wkT", [HID, NKV * D])
    wvT = param("wvT", [HID, NKV * D])
    woT = param("woT", [HID, HID])
    bqT_d = param("bqT", [D, NH], F32)
    bkT_d = param("bkT", [D, NKV], F32)
    bv_d = param("bv", [1, NKV * D])
    cosT_d = param("cosT", [D, S])
    sinT_d = param("sinT", [D, S])
    cq_d = param("cosTq", [D, NQ])
    sq_d = param("sinTq", [D, NQ])
    maskT_d = param("maskT", [S, NQ])     # exp(mask).T, fp32r-rounded
    out_d = nc.declare_dram_parameter("out", [NQ, HID], F32, isOutput=True)[:]

    HC = HID // 128   # 16 contraction chunks
    KT = S // 128     # 16 key tiles
    KT2 = KT // 2     # 8 key tile-pairs
    TC = S // NQ      # 4 token chunks (for K/V proj)
    QS = NQ // 128    # 4 query sub-tiles

    Exp = mybir.ActivationFunctionType.Exp
    Ident = mybir.ActivationFunctionType.Identity

    lp = (nc.allow_low_precision(reason="fp32r matmul operands; psum stays f32")
          if mm == "f32r" else contextlib.nullcontext())
    with lp, tile.TileContext(nc) as tc:
        with tc.tile_pool(name="const", bufs=1) as cst, \
             tc.tile_pool(name="maskp", bufs=1) as maskp, \
             tc.tile_pool(name="kvp", bufs=1) as kvp:

            ones_row = cst.tile([1, 128], MMDT, name="ones_row")
            ones_sq = cst.tile([128, 128], MMDT, name="ones_sq")
            ones_f32 = cst.tile([128, 128], F32, name="ones_f32")
            nc.vector.memset(ones_f32, 1.0)
            nc.vector.tensor_copy(ones_row, ones_f32[0:1, :])
            nc.vector.tensor_copy(ones_sq, ones_f32)
            bqT = cst.tile([D, NH], F32, name="bqT")
            bkT = cst.tile([D, NKV], F32, name="bkT")
            bvr = cst.tile([1, NKV * D], MMDT, name="bvr")
            nc.sync.dma_start(bqT, bqT_d)
            nc.sync.dma_start(bkT, bkT_d)
            nc.sync.dma_start(bvr, bv_d)

            # exp(mask) tiles [128 k, 2 kt, 512 q], resident through attention
            mask_sb = [maskp.tile([128, 2, NQ], MMDT, name=f"mask{kt}")
                       for kt in range(KT2)]

            # token chunk 0 of xT = this core's query columns (host permutes
            # chunks); kept resident for the Q projection
            xq_sb = [kvp.tile([128, NQ], MMDT, name=f"xq{c}")
                     for c in range(HC)]
            # persistent K^T [d, t] per kv head; V [t, d] per token tile
            kT_sb = [kvp.tile([128, S], MMDT, name=f"kT{g}")
                     for g in range(NKV)]
            v_sb = [kvp.tile([128, NKV * D], MMDT, name=f"v{t}")
                    for t in range(KT)]

            # ---------------- P1a: K/V projection over all tokens ----------
            with tc.tile_pool(name="p1", bufs=1) as p1, \
                 tc.tile_pool(name="p1s", bufs=3) as p1s, \
                 tc.tile_pool(name="p1ps", bufs=1, space="PSUM") as p1ps:
                wk_sb = [p1.tile([128, NKV * D], MMDT, name=f"wk{c}")
                         for c in range(HC)]
                wv_sb = [p1.tile([128, NKV * D], MMDT, name=f"wv{c}")
                         for c in range(HC)]

                for tch in range(TC):
                    tsl = slice(tch * NQ, (tch + 1) * NQ)
                    kps = [p1ps.tile([128, NQ], F32, name=f"kps{g}", bufs=2)
                           for g in range(NKV)]
                    vps = [p1ps.tile([128, NKV * D], F32, name=f"vps{s_}",
                                     bufs=1) for s_ in range(4)]
                    for c in range(HC):
                        if tch == 0:
                            nc.sync.dma_start(wk_sb[c],
                                              wkT[c * 128:(c + 1) * 128, :])
                            nc.sync.dma_start(wv_sb[c],
                                              wvT[c * 128:(c + 1) * 128, :])
                            xt = xq_sb[c]
                        else:
                            xt = p1s.tile([128, NQ], MMDT, name="xt")
                        nc.sync.dma_start(xt, xT[c * 128:(c + 1) * 128, tsl])
                        for g in range(NKV):
                            nc.tensor.matmul(
                                kps[g], wk_sb[c][:, g * D:(g + 1) * D],
                                xt, start=(c == 0), stop=(c == HC - 1))
                        for s_ in range(4):
                            nc.tensor.matmul(
                                vps[s_], xt[:, s_ * 128:(s_ + 1) * 128],
                                wv_sb[c], start=(c == 0), stop=False)
                    # V bias via K=1 ones matmul, then evacuate
                    for s_ in range(4):
                        nc.tensor.matmul(vps[s_], ones_row, bvr,
                                         start=False, stop=True)
                        nc.vector.tensor_copy(v_sb[tch * 4 + s_], vps[s_])
                    # K bias + rope -> kT_sb
                    csb = p1s.tile([128, NQ], MMDT, name="csb")
                    ssb = p1s.tile([128, NQ], MMDT, name="ssb")
                    nc.sync.dma_start(csb, cosT_d[:, tsl])
                    nc.sync.dma_start(ssb, sinT_d[:, tsl])
                    for g in range(NKV):
                        kb = p1s.tile([128, NQ], MMDT, name="kb")
                        nc.scalar.activation(kb, kps[g], Ident,
                                             bias=bkT[:, g:g + 1])
                        ke = kT_sb[g][:, tsl]
                        shuf = p1s.tile([128, NQ], MMDT, name="shuf")
                        nc.sync.dma_start(shuf[0:64, :], kb[64:128, :])
                        nc.sync.dma_start(shuf[64:128, :], kb[0:64, :])
                        nc.vector.tensor_mul(ke, kb, csb)
                        nc.vector.tensor_mul(shuf, shuf, ssb)
                        nc.vector.tensor_add(ke, ke, shuf)

            # -------- P1b + P2: Q proj interleaved with attention ----------
            for kt2 in range(KT2):
                nc.sync.dma_start(
                    mask_sb[kt2],
                    maskT_d[256 * kt2:256 * (kt2 + 1), :].rearrange(
                        "(a p) q -> p a q", a=2))
            with tc.tile_pool(name="ap", bufs=1) as ap:
                a_sb = [ap.tile([128, NQ], MMDT, name=f"a{h}")
                        for h in range(NH)]
                with tc.tile_pool(name="p2", bufs=1) as p2, \
                     tc.tile_pool(name="p2s", bufs=2) as p2s, \
                     tc.tile_pool(name="p2w", bufs=5) as p2w, \
                     tc.tile_pool(name="qtp", bufs=2) as qtp, \
                     tc.tile_pool(name="att", bufs=5) as att, \
                     tc.tile_pool(name="atts", bufs=2) as atts:
                    cq = p2.tile([D, NQ], MMDT, name="cq")
                    sq = p2.tile([D, NQ], MMDT, name="sq")
                    nc.sync.dma_start(cq, cq_d)
                    nc.sync.dma_start(sq, sq_d)

                    for hg in range(4):
                        qT_sb = {}
                        with tc.tile_pool(name=f"qps{hg}", bufs=1,
                                          space="PSUM") as p2ps:
                            qps = [p2ps.tile([128, NQ], F32, name=f"qps{j}",
                                             bufs=1) for j in range(4)]
                            for c in range(HC):
                                wq = p2w.tile([128, NQ], MMDT, name="wq")
                                nc.sync.dma_start(
                                    wq, wqT[c * 128:(c + 1) * 128,
                                            hg * NQ:(hg + 1) * NQ])
                                for j in range(4):
                                    nc.tensor.matmul(
                                        qps[j], wq[:, j * 128:(j + 1) * 128],
                                        xq_sb[c], start=(c == 0),
                                        stop=(c == HC - 1))
                            for j in range(4):
                                h = hg * 4 + j
                                qT_sb[h] = qtp.tile([128, NQ], MMDT,
                                                    name=f"qT{j}")
                                qb = p2s.tile([128, NQ], MMDT, name="qb")
                                nc.scalar.activation(qb, qps[j], Ident,
                                                     bias=bqT[:, h:h + 1])
                                qe = qT_sb[h]
                                shufq = p2s.tile([128, NQ], MMDT,
                                                 name="shufq")
                                nc.sync.dma_start(shufq[0:64, :],
                                                  qb[64:128, :])
                                nc.sync.dma_start(shufq[64:128, :],
                                                  qb[0:64, :])
                                nc.vector.tensor_mul(qe, qb, cq)
                                nc.vector.tensor_mul(shufq, shufq, sq)
                                nc.vector.tensor_add(qe, qe, shufq)

                        with tc.tile_pool(name=f"attps{hg}", bufs=1,
                                          space="PSUM") as attps:
                            for h in range(hg * 4, hg * 4 + 4):
                                g = h // (NH // NKV)
                                ops = attps.tile([128, NQ], F32, name="ops",
                                                 bufs=1)
                                stats = attps.tile([128, NQ], F32,
                                                   name="stats", bufs=1)
                                for kt2 in range(KT2):
                                    sps = attps.tile([128, 2, NQ], F32,
                                                     name="sps", bufs=3)
                                    ebuf = att.tile([128, 2, NQ], MMDT,
                                                    name="ebuf")
                                    for j2 in range(2):
                                        kt = 2 * kt2 + j2
                                        nc.tensor.matmul(
                                            sps[:, j2, :],
                                            kT_sb[g][:, kt * 128:
                                                     (kt + 1) * 128],
                                            qT_sb[h], start=True, stop=True)
                                    nc.scalar.activation(
                                        ebuf.rearrange("p a b -> p (a b)"),
                                        sps.rearrange("p a b -> p (a b)"),
                                        Exp, scale=float(SM_SCALE))
                                    nc.vector.tensor_mul(
                                        ebuf.rearrange("p a b -> p (a b)"),
                                        ebuf.rearrange("p a b -> p (a b)"),
                                        mask_sb[kt2].rearrange(
                                            "p a b -> p (a b)"))
                                    for j2 in range(2):
                                        kt = 2 * kt2 + j2
                                        nc.tensor.matmul(
                                            stats, ones_sq, ebuf[:, j2, :],
                                            start=(kt == 0),
                                            stop=(kt == KT - 1))
                                        nc.tensor.matmul(
                                            ops,
                                            v_sb[kt][:, g * D:(g + 1) * D],
                                            ebuf[:, j2, :],
                                            start=(kt == 0),
                                            stop=(kt == KT - 1))
                                recip = atts.tile([128, NQ], F32,
                                                  name="recip")
                                nc.vector.reciprocal_approx_fast(
                                    out=recip, in_=stats)
                                nc.vector.tensor_mul(a_sb[h], ops, recip)

                # ------------- P3: o_proj ------------------------------
                with tc.tile_pool(name="wop", bufs=1) as wop, \
                     tc.tile_pool(name="wos", bufs=3) as wos, \
                     tc.tile_pool(name="wops", bufs=1, space="PSUM") as wops:
                    for ec in range(4):
                        wo_t = [wop.tile([128, NQ], MMDT, name=f"wo{h}",
                                         bufs=2) for h in range(NH)]
                        for h in range(NH):
                            nc.sync.dma_start(
                                wo_t[h], woT[h * 128:(h + 1) * 128,
                                             ec * NQ:(ec + 1) * NQ])
                        for qs_ in range(QS):
                            opo = wops.tile([128, NQ], F32, name="opo",
                                            bufs=3)
                            for h in range(NH):
                                nc.tensor.matmul(
                                    opo,
                                    a_sb[h][:, qs_ * 128:(qs_ + 1) * 128],
                                    wo_t[h], start=(h == 0),
                                    stop=(h == NH - 1))
                            osb = wos.tile([128, NQ], F32, name="osb")
                            nc.vector.tensor_copy(osb, opo)
                            nc.sync.dma_start(
                                out_d[qs_ * 128:(qs_ + 1) * 128,
                                      ec * NQ:(ec + 1) * NQ], osb)
    return nc


def get_nc(mm="f32r"):
    if mm not in _BUILD_CACHE:
        nc = _build_nc(mm)
        nc.finalize()
        _BUILD_CACHE[mm] = nc
    return _BUILD_CACHE[mm]


_MROPE_SECTION = [16, 24, 24]
_STREAM_IDX = np.concatenate(
    [np.full(n, i % 3, np.int64)
     for i, n in enumerate(_MROPE_SECTION * 2)])  # [128]


def _host_prep(hidden_states, cos, sin, attention_mask, Wq, bq, Wk, bk, Wv,
               bv, Wo, mm="f32r"):
    f = np.float32
    if mm == "f32r":
        rnd = _round_fp32r
    else:
        def rnd(a):
            return np.ascontiguousarray(a, f)
    hs = np.asarray(hidden_states, f)
    cos = np.asarray(cos, f)
    sin = np.asarray(sin, f)
    mask = np.asarray(attention_mask, f)
    ar = np.arange(D)

    shared = {
        "wqT": rnd(np.asarray(Wq, f).T),
        "wkT": rnd(np.asarray(Wk, f).T),
        "wvT": rnd(np.asarray(Wv, f).T),
        "woT": rnd(np.asarray(Wo, f).T),
        "bqT": np.ascontiguousarray(np.asarray(bq, f).reshape(NH, D).T),
        "bkT": np.ascontiguousarray(np.asarray(bk, f).reshape(NKV, D).T),
        "bv": rnd(np.asarray(bv, f).reshape(1, NKV * D)),
    }

    per_batch = []
    for b in range(B):
        xT = rnd(hs[b].T)
        cosT = rnd(cos[_STREAM_IDX, b, :, ar])  # [128, S]
        sinT = rnd(sin[_STREAM_IDX, b, :, ar])
        sinT[0:64, :] *= -1.0   # rotate_half sign folded into sin
        maskT = rnd(np.exp(mask[b, 0].T.astype(np.float64)
                           ).astype(np.float32))
        per_batch.append((xT, cosT, sinT, maskT))

    in_maps = []
    for c in range(N_CORES):
        b, qc = divmod(c, N_CORES // B)
        xT, cosT, sinT, maskT = per_batch[b]
        qsl = slice(qc * NQ, (qc + 1) * NQ)
        order = [qc] + [o for o in range(N_CORES // B) if o != qc]
        tperm = np.concatenate([np.arange(o * NQ, (o + 1) * NQ)
                                for o in order])
        m = dict(shared)
        m["xT"] = np.ascontiguousarray(xT[:, tperm])
        m["cosT"] = np.ascontiguousarray(cosT[:, tperm])
        m["sinT"] = np.ascontiguousarray(sinT[:, tperm])
        m["maskT"] = np.ascontiguousarray(maskT[tperm][:, qsl])
        m["cosTq"] = np.ascontiguousarray(cosT[:, qsl])
        m["sinTq"] = np.ascontiguousarray(sinT[:, qsl])
        in_maps.append(m)
    return in_maps


def kernel(hidden_states, cos, sin, attention_mask, Wq, bq, Wk, bk, Wv, bv,
           Wo, _trace=False, _mm="f32r"):
    from concourse.bass_utils import run_bass_kernel_spmd

    in_maps = _host_prep(hidden_states, cos, sin, attention_mask, Wq, bq, Wk,
                         bk, Wv, bv, Wo, mm=_mm)
    nc = get_nc(_mm)
    res = run_bass_kernel_spmd(nc, in_maps, list(range(N_CORES)),
                               trace=_trace)
    out = np.empty((B, S, HID), np.float32)
    for c in range(N_CORES):
        b, qc = divmod(c, N_CORES // B)
        out[b, qc * NQ:(qc + 1) * NQ, :] = res.results[c]["out"]
    kernel._last_results = res
    return out


# revision 14
# speedup vs baseline: 1.5665x; 1.0573x over previous
"""Qwen2.5-VL attention (mrope + GQA + causal mask + o_proj) on 8 Trainium2
NeuronCores.

Sharding: batch x query-chunk. Core c handles batch b = c//4 and query rows
[512*(c%4), 512*(c%4)+512). Each core computes K/V projections for all 2048
tokens of its batch, Q projection + full attention + o_proj for its 512 query
rows, and writes a [512, 2048] output slice. Host concatenates - no
cross-core reduction.

On-device layout: everything transposed so the PE contraction dim is always
on partitions.  Host pre-transposes hidden (xT), weights (wqT/wkT/wvT/woT),
merged-mrope cos/sin, and the mask slice.
  - QT/KT produced as [d, t]; scores computed transposed S^T[k, q]
  - exp on ScalarE straight from PSUM with the 1/sqrt(D) scale folded in;
    additive mask applied as elementwise multiply by host-precomputed
    exp(mask) (exact 0/1 for a causal mask)
  - softmax denominators via ones[128,128] matmuls (sums arrive broadcast
    across partitions), normalization = reciprocal + multiply
  - PV accumulates outT[d, q]; o_proj consumes outT directly as lhsT
  - Q projection is interleaved with attention per head group so the wq
    weight stream hides behind attention compute

Matmuls run in fp32r (fp32 with 12-bit mantissa rounding, 4x faster than
plain fp32 on the PE).  Host pre-rounds all DMA-fed matmul operands; compute
ops that produce matmul operands write fp32r tiles (HW rounds on write).
"""

import sys

for _p in ("/opt/trn_rl_repo", "/root/.axon_site/_ro/trn_rl_repo"):
    if _p not in sys.path:
        sys.path.insert(0, _p)

import numpy as np

B = 2
S = 2048
HID = 2048
NH = 16
NKV = 2
D = 128
NQ = 512          # query rows per core
N_CORES = 8
SM_SCALE = 1.0 / np.sqrt(np.float32(D))

_BUILD_CACHE = {}


def _round_fp32r(a):
    """Round-to-nearest-even to 12 explicit mantissa bits (fp32r)."""
    u = np.ascontiguousarray(a, np.float32).view(np.uint32)
    low = u & np.uint32(0xFFF)
    up = (u & np.uint32(0xFFFFF000)) + np.uint32(0x1000)
    half = low == np.uint32(0x800)
    rnd = np.where(low > 0x800, up,
                   np.where(half & ((u & np.uint32(0x1000)) != 0), up,
                            u & np.uint32(0xFFFFF000)))
    expmask = (u & np.uint32(0x7F800000)) == np.uint32(0x7F800000)
    rnd = np.where(expmask, u, rnd)
    return rnd.view(np.float32)


def _build_nc(mm="f32r"):
    import contextlib
    import concourse.bass as bass
    import concourse.tile as tile
    from concourse import bacc, mybir

    F32 = mybir.dt.float32
    MMDT = mybir.dt.float32r if mm == "f32r" else F32

    nc = bacc.Bacc(target_bir_lowering=False, debug=False)

    def param(name, shape, dt=MMDT):
        return nc.declare_dram_parameter(name, list(shape), dt,
                                         isOutput=False)[:]

    xT = param("xT", [HID, S])
    wqT = param("wqT", [HID, HID])
    wkT = param("wkT", [HID, NKV * D])
    wvT = param("wvT", [HID, NKV * D])
    woT = param("woT", [HID, HID])
    bqT_d = param("bqT", [D, NH], F32)
    bkT_d = param("bkT", [D, NKV], F32)
    bv_d = param("bv", [1, NKV * D])
    cosT_d = param("cosT", [D, S])
    sinT_d = param("sinT", [D, S])
    cq_d = param("cosTq", [D, NQ])
    sq_d = param("sinTq", [D, NQ])
    maskT_d = param("maskT", [S, NQ])     # exp(mask).T, fp32r-rounded
    out_d = nc.declare_dram_parameter("out", [NQ, HID], F32, isOutput=True)[:]

    HC = HID // 128   # 16 contraction chunks
    KT = S // 128     # 16 key tiles
    KT2 = KT // 2     # 8 key tile-pairs
    TC = S // NQ      # 4 token chunks (for K/V proj)
    QS = NQ // 128    # 4 query sub-tiles

    Exp = mybir.ActivationFunctionType.Exp
    Ident = mybir.ActivationFunctionType.Identity

    lp = (nc.allow_low_precision(reason="fp32r matmul operands; psum stays f32")
          if mm == "f32r" else contextlib.nullcontext())
    with lp, tile.TileContext(nc) as tc:
        with tc.tile_pool(name="const", bufs=1) as cst, \
             tc.tile_pool(name="maskp", bufs=1) as maskp, \
             tc.tile_pool(name="kvp", bufs=1) as kvp:

            ones_row = cst.tile([1, 128], MMDT, name="ones_row")
            ones_sq = cst.tile([128, 128], MMDT, name="ones_sq")
            ones_f32 = cst.tile([128, 128], F32, name="ones_f32")
            nc.vector.memset(ones_f32, 1.0)
            nc.vector.tensor_copy(ones_row, ones_f32[0:1, :])
            nc.vector.tensor_copy(ones_sq, ones_f32)
            bqT = cst.tile([D, NH], F32, name="bqT")
            bkT = cst.tile([D, NKV], F32, name="bkT")
            bvr = cst.tile([1, NKV * D], MMDT, name="bvr")
            nc.sync.dma_start(bqT, bqT_d)
            nc.sync.dma_start(bkT, bkT_d)
            nc.sync.dma_start(bvr, bv_d)

            # exp(mask) tiles [128 k, 2 kt, 512 q], resident through attention
            mask_sb = [maskp.tile([128, 2, NQ], MMDT, name=f"mask{kt}")
                       for kt in range(KT2)]

            # token chunk 0 of xT = this core's query columns (host permutes
            # chunks); kept resident for the Q projection
            xq_sb = [kvp.tile([128, NQ], MMDT, name=f"xq{c}")
                     for c in range(HC)]
            # persistent K^T [d, t] per kv head; V [t, d] per token tile
            kT_sb = [kvp.tile([128, S], MMDT, name=f"kT{g}")
                     for g in range(NKV)]
            v_sb = [kvp.tile([128, NKV * D], MMDT, name=f"v{t}")
                    for t in range(KT)]

            # ---------------- P1a: K/V projection over all tokens ----------
            with tc.tile_pool(name="p1", bufs=1) as p1, \
                 tc.tile_pool(name="p1s", bufs=3) as p1s, \
                 tc.tile_pool(name="p1ps", bufs=1, space="PSUM") as p1ps:
                wk_sb = [p1.tile([128, NKV * D], MMDT, name=f"wk{c}")
                         for c in range(HC)]
                wv_sb = [p1.tile([128, NKV * D], MMDT, name=f"wv{c}")
                         for c in range(HC)]

                for tch in range(TC):
                    tsl = slice(tch * NQ, (tch + 1) * NQ)
                    kps = [p1ps.tile([128, NQ], F32, name=f"kps{g}", bufs=2)
                           for g in range(NKV)]
                    vps = [p1ps.tile([128, NKV * D], F32, name=f"vps{s_}",
                                     bufs=1) for s_ in range(4)]
                    for c in range(HC):
                        if tch == 0:
                            nc.sync.dma_start(wk_sb[c],
                                              wkT[c * 128:(c + 1) * 128, :])
                            nc.sync.dma_start(wv_sb[c],
                                              wvT[c * 128:(c + 1) * 128, :])
                            xt = xq_sb[c]
                        else:
                            xt = p1s.tile([128, NQ], MMDT, name="xt",
                                          bufs=8)
                        nc.sync.dma_start(xt, xT[c * 128:(c + 1) * 128, tsl])
                        for g in range(NKV):
                            nc.tensor.matmul(
                                kps[g], wk_sb[c][:, g * D:(g + 1) * D],
                                xt, start=(c == 0), stop=(c == HC - 1))
                        for s_ in range(4):
                            nc.tensor.matmul(
                                vps[s_], xt[:, s_ * 128:(s_ + 1) * 128],
                                wv_sb[c], start=(c == 0), stop=False)
                    # V bias via K=1 ones matmul, then evacuate
                    for s_ in range(4):
                        nc.tensor.matmul(vps[s_], ones_row, bvr,
                                         start=False, stop=True)
                        nc.vector.tensor_copy(v_sb[tch * 4 + s_], vps[s_])
                    # K bias + rope -> kT_sb
                    csb = p1s.tile([128, NQ], MMDT, name="csb")
                    ssb = p1s.tile([128, NQ], MMDT, name="ssb")
                    nc.sync.dma_start(csb, cosT_d[:, tsl])
                    nc.sync.dma_start(ssb, sinT_d[:, tsl])
                    for g in range(NKV):
                        kb = p1s.tile([128, NQ], MMDT, name="kb")
                        nc.scalar.activation(kb, kps[g], Ident,
                                             bias=bkT[:, g:g + 1])
                        ke = kT_sb[g][:, tsl]
                        shuf = p1s.tile([128, NQ], MMDT, name="shuf")
                        nc.sync.dma_start(shuf[0:64, :], kb[64:128, :])
                        nc.sync.dma_start(shuf[64:128, :], kb[0:64, :])
                        nc.vector.tensor_mul(ke, kb, csb)
                        nc.vector.tensor_mul(shuf, shuf, ssb)
                        nc.vector.tensor_add(ke, ke, shuf)

            # -------- P1b + P2: Q proj interleaved with attention ----------
            for kt2 in range(KT2):
                nc.sync.dma_start(
                    mask_sb[kt2],
                    maskT_d[256 * kt2:256 * (kt2 + 1), :].rearrange(
                        "(a p) q -> p a q", a=2))
            with tc.tile_pool(name="ap", bufs=1) as ap:
                a_sb = [ap.tile([128, NQ], MMDT, name=f"a{h}")
                        for h in range(NH)]
                with tc.tile_pool(name="p2", bufs=1) as p2, \
                     tc.tile_pool(name="p2s", bufs=2) as p2s, \
                     tc.tile_pool(name="p2w", bufs=5) as p2w, \
                     tc.tile_pool(name="qtp", bufs=2) as qtp, \
                     tc.tile_pool(name="att", bufs=5) as att, \
                     tc.tile_pool(name="atts", bufs=2) as atts:
                    cq = p2.tile([D, NQ], MMDT, name="cq")
                    sq = p2.tile([D, NQ], MMDT, name="sq")
                    nc.sync.dma_start(cq, cq_d)
                    nc.sync.dma_start(sq, sq_d)

                    for hg in range(4):
                        qT_sb = {}
                        with tc.tile_pool(name=f"qps{hg}", bufs=1,
                                          space="PSUM") as p2ps:
                            qps = [p2ps.tile([128, NQ], F32, name=f"qps{j}",
                                             bufs=1) for j in range(4)]
                            for c in range(HC):
                                wq = p2w.tile([128, NQ], MMDT, name="wq")
                                nc.sync.dma_start(
                                    wq, wqT[c * 128:(c + 1) * 128,
                                            hg * NQ:(hg + 1) * NQ])
                                for j in range(4):
                                    nc.tensor.matmul(
                                        qps[j], wq[:, j * 128:(j + 1) * 128],
                                        xq_sb[c], start=(c == 0),
                                        stop=(c == HC - 1))
                            for j in range(4):
                                h = hg * 4 + j
                                qT_sb[h] = qtp.tile([128, NQ], MMDT,
                                                    name=f"qT{j}")
                                qb = p2s.tile([128, NQ], MMDT, name="qb")
                                nc.scalar.activation(qb, qps[j], Ident,
                                                     bias=bqT[:, h:h + 1])
                                qe = qT_sb[h]
                                shufq = p2s.tile([128, NQ], MMDT,
                                                 name="shufq")
                                nc.sync.dma_start(shufq[0:64, :],
                                                  qb[64:128, :])
                                nc.sync.dma_start(shufq[64:128, :],
                                                  qb[0:64, :])
                                nc.vector.tensor_mul(qe, qb, cq)
                                nc.vector.tensor_mul(shufq, shufq, sq)
                                nc.vector.tensor_add(qe, qe, shufq)

                        with tc.tile_pool(name=f"attps{hg}", bufs=1,
                                          space="PSUM") as attps:
                            for h in range(hg * 4, hg * 4 + 4):
                                g = h // (NH // NKV)
                                ops = attps.tile([128, NQ], F32, name="ops",
                                                 bufs=1)
                                stats = attps.tile([128, NQ], F32,
                                                   name="stats", bufs=1)
                                for kt2 in range(KT2):
                                    sps = attps.tile([128, 2, NQ], F32,
                                                     name="sps", bufs=3)
                                    ebuf = att.tile([128, 2, NQ], MMDT,
                                                    name="ebuf")
                                    for j2 in range(2):
                                        kt = 2 * kt2 + j2
                                        nc.tensor.matmul(
                                            sps[:, j2, :],
                                            kT_sb[g][:, kt * 128:
                                                     (kt + 1) * 128],
                                            qT_sb[h], start=True, stop=True)
                                    nc.scalar.activation(
                                        ebuf.rearrange("p a b -> p (a b)"),
                                        sps.rearrange("p a b -> p (a b)"),
                                        Exp, scale=float(SM_SCALE))
                                    nc.vector.tensor_mul(
                                        ebuf.rearrange("p a b -> p (a b)"),
                                        ebuf.rearrange("p a b -> p (a b)"),
                                        mask_sb[kt2].rearrange(
                                            "p a b -> p (a b)"))
                                    for j2 in range(2):
                                        kt = 2 * kt2 + j2
                                        nc.tensor.matmul(
                                            stats, ones_sq, ebuf[:, j2, :],
                                            start=(kt == 0),
                                            stop=(kt == KT - 1))
                                        nc.tensor.matmul(
                                            ops,
                                            v_sb[kt][:, g * D:(g + 1) * D],
                                            ebuf[:, j2, :],
                                            start=(kt == 0),
                                            stop=(kt == KT - 1))
                                recip = atts.tile([128, NQ], F32,
                                                  name="recip")
                                nc.vector.reciprocal_approx_fast(
                                    out=recip, in_=stats)
                                nc.vector.tensor_mul(a_sb[h], ops, recip)

                # ------------- P3: o_proj ------------------------------
                with tc.tile_pool(name="wop", bufs=1) as wop, \
                     tc.tile_pool(name="wos", bufs=3) as wos, \
                     tc.tile_pool(name="wops", bufs=1, space="PSUM") as wops:
                    for ec in range(4):
                        wo_t = [wop.tile([128, NQ], MMDT, name=f"wo{h}",
                                         bufs=2) for h in range(NH)]
                        for h in range(NH):
                            nc.sync.dma_start(
                                wo_t[h], woT[h * 128:(h + 1) * 128,
                                             ec * NQ:(ec + 1) * NQ])
                        for qs_ in range(QS):
                            opo = wops.tile([128, NQ], F32, name="opo",
                                            bufs=3)
                            for h in range(NH):
                                nc.tensor.matmul(
                                    opo,
                                    a_sb[h][:, qs_ * 128:(qs_ + 1) * 128],
                                    wo_t[h], start=(h == 0),
                                    stop=(h == NH - 1))
                            osb = wos.tile([128, NQ], F32, name="osb")
                            nc.vector.tensor_copy(osb, opo)
                            nc.sync.dma_start(
                                out_d[qs_ * 128:(qs_ + 1) * 128,
                                      ec * NQ:(ec + 1) * NQ], osb)
    return nc


def get_nc(mm="f32r"):
    if mm not in _BUILD_CACHE:
        nc = _build_nc(mm)
        nc.finalize()
        _BUILD_CACHE[mm] = nc
    return _BUILD_CACHE[mm]


_MROPE_SECTION = [16, 24, 24]
_STREAM_IDX = np.concatenate(
    [np.full(n, i % 3, np.int64)
     for i, n in enumerate(_MROPE_SECTION * 2)])  # [128]


def _host_prep(hidden_states, cos, sin, attention_mask, Wq, bq, Wk, bk, Wv,
               bv, Wo, mm="f32r"):
    f = np.float32
    if mm == "f32r":
        rnd = _round_fp32r
    else:
        def rnd(a):
            return np.ascontiguousarray(a, f)
    hs = np.asarray(hidden_states, f)
    cos = np.asarray(cos, f)
    sin = np.asarray(sin, f)
    mask = np.asarray(attention_mask, f)
    ar = np.arange(D)

    shared = {
        "wqT": rnd(np.asarray(Wq, f).T),
        "wkT": rnd(np.asarray(Wk, f).T),
        "wvT": rnd(np.asarray(Wv, f).T),
        "woT": rnd(np.asarray(Wo, f).T),
        "bqT": np.ascontiguousarray(np.asarray(bq, f).reshape(NH, D).T),
        "bkT": np.ascontiguousarray(np.asarray(bk, f).reshape(NKV, D).T),
        "bv": rnd(np.asarray(bv, f).reshape(1, NKV * D)),
    }

    per_batch = []
    for b in range(B):
        xT = rnd(hs[b].T)
        cosT = rnd(cos[_STREAM_IDX, b, :, ar])  # [128, S]
        sinT = rnd(sin[_STREAM_IDX, b, :, ar])
        sinT[0:64, :] *= -1.0   # rotate_half sign folded into sin
        maskT = rnd(np.exp(mask[b, 0].T.astype(np.float64)
                           ).astype(np.float32))
        per_batch.append((xT, cosT, sinT, maskT))

    in_maps = []
    for c in range(N_CORES):
        b, qc = divmod(c, N_CORES // B)
        xT, cosT, sinT, maskT = per_batch[b]
        qsl = slice(qc * NQ, (qc + 1) * NQ)
        order = [qc] + [o for o in range(N_CORES // B) if o != qc]
        tperm = np.concatenate([np.arange(o * NQ, (o + 1) * NQ)
                                for o in order])
        m = dict(shared)
        m["xT"] = np.ascontiguousarray(xT[:, tperm])
        m["cosT"] = np.ascontiguousarray(cosT[:, tperm])
        m["sinT"] = np.ascontiguousarray(sinT[:, tperm])
        m["maskT"] = np.ascontiguousarray(maskT[tperm][:, qsl])
        m["cosTq"] = np.ascontiguousarray(cosT[:, qsl])
        m["sinTq"] = np.ascontiguousarray(sinT[:, qsl])
        in_maps.append(m)
    return in_maps


def kernel(hidden_states, cos, sin, attention_mask, Wq, bq, Wk, bk, Wv, bv,
           Wo, _trace=False, _mm="f32r"):
    from concourse.bass_utils import run_bass_kernel_spmd

    in_maps = _host_prep(hidden_states, cos, sin, attention_mask, Wq, bq, Wk,
                         bk, Wv, bv, Wo, mm=_mm)
    nc = get_nc(_mm)
    res = run_bass_kernel_spmd(nc, in_maps, list(range(N_CORES)),
                               trace=_trace)
    out = np.empty((B, S, HID), np.float32)
    for c in range(N_CORES):
        b, qc = divmod(c, N_CORES // B)
        out[b, qc * NQ:(qc + 1) * NQ, :] = res.results[c]["out"]
    kernel._last_results = res
    return out
